# revision 13
# baseline (speedup 1.0000x reference)
"""3D Haar DWT (single level) on 8 Trainium2 NeuronCores.

Input x: (2, 4, 128, 256, 256) f32. Output: 8 subbands (LLL..HHH), each
(2, 4, 64, 128, 128).

Sharding: pure data parallel - B*C = 8 independent (128, 256, 256) volumes,
one per core. No cross-core communication.

Per-core layout: partitions = D (128 d-slices), so every DMA run is a fully
contiguous 16 KiB per partition. Data path is fp16 (tolerance 2e-2, fp16
round-off ~1e-3).

Per h-chunk (32 rows -> 16 output rows), 8 chunks per volume:
  DMA in  : X[p=d, (h32 w256)] fp16, 16KiB/partition contig   (SP HWDGE)
  DVE     : W-axis pairs  -> Wb[p, (kW h32 r128)]             (2 TT)
  Pool    : H-axis pairs  -> Hb[p, (kH kW q16 r128)]          (2 TT)
  PE      : D-axis Haar matmul, lhsT = +-1 matrix fp16        (16 matmuls)
  ScalarE : psum -> SBUF fp16 copy, scale=(1/sqrt2)^3 folded  (4 copies)
  DMA out : Y[p, 8192] -> y[p, c, :] 16KiB/partition contig   (ACT HWDGE)
Host: cast x to fp16, reassemble y into the 8 subbands in f32.
"""

import sys

sys.path.insert(0, "/opt/trn_rl_repo")

import json

import numpy as np

import concourse.bass as bass
import concourse.mybir as mybir
import concourse.tile as tile
from concourse import bass_utils

_C3 = np.float32(1.0 / (2.0 * np.sqrt(2.0)))  # (1/sqrt2)^3, folded into Act copy

# ---------------------------------------------------------------------------
# BIR post-pass: this walrus build has tight per-instruction sync-wait
# encoding limits (Drain/TPB_CTRL: 0 waits; everything else observed to
# reject 2+ waits: Matmult/S3_LW, DMACopy, TensorTensor). Keep at most one
# wait per instruction and hoist the excess onto EventSemaphore instructions
# inserted right before it on the same engine - program order makes that
# equivalent.
# ---------------------------------------------------------------------------
_MAX_WAITS = {"Drain": 0}
_DEFAULT_MAX_WAITS = 1


def _fix_sync_limits(bir_bytes: bytes) -> bytes:
    m = json.loads(bir_bytes)

    def fix_block(blk):
        insts = blk.get("instructions", [])
        new = []
        for i in insts:
            limit = _MAX_WAITS.get(i.get("opcode"), _DEFAULT_MAX_WAITS)
            si = i.get("sync_info") or {}
            waits = si.get("on_wait") or []
            if len(waits) > limit:
                n_hoist = len(waits) - limit
                for wi, w in enumerate(waits[:n_hoist]):
                    ev = {
                        "name": i["name"] + f"-hoistwait{wi}",
                        "opcode": "EventSemaphore",
                        "engine": i["engine"],
                        "ins": [],
                        "outs": [],
                        "sync_info": {"on_wait": [w], "on_update": []},
                    }
                    if "debug" in i:
                        ev["debug"] = i["debug"]
                    new.append(ev)
                si = dict(si)
                si["on_wait"] = waits[n_hoist:]
                i = dict(i)
                i["sync_info"] = si
            new.append(i)
        blk["instructions"] = new
        for sub in blk.get("blocks", []):
            fix_block(sub)

    for f in m["functions"]:
        for blk in f["blocks"]:
            fix_block(blk)
    return json.dumps(m).encode()


_patched = False


def _install_patch():
    global _patched
    if _patched:
        return
    orig = bass.Bass.to_json_bytes

    def patched(self, *a, **k):
        return _fix_sync_limits(orig(self, *a, **k))

    bass.Bass.to_json_bytes = patched
    _patched = True


def _build_haar_matrix() -> np.ndarray:
    """lhsT [d, m'=(kD*64 + mu)]: D-axis Haar pairs with the full 3-axis
    scale folded in (+-C3; fp16 rounding of C3 is a uniform 1.5e-4 scale
    error). The psum->SBUF copies are then pure f32->f16 casts."""
    c = np.float16(_C3)
    M = np.zeros((128, 128), np.float16)
    for mu in range(64):
        M[2 * mu, mu] = c
        M[2 * mu + 1, mu] = c
        M[2 * mu, 64 + mu] = c
        M[2 * mu + 1, 64 + mu] = -c
    return M


_PROGRAM = None


def _build_program(reps: int = 1) -> bass.Bass:
    """reps>1 wraps the whole pipeline in a dynamic loop (benchmarking only)."""
    global _PROGRAM
    if reps == 1 and _PROGRAM is not None:
        return _PROGRAM
    _install_patch()

    F16 = mybir.dt.float16
    F32 = mybir.dt.float32
    nc = bass.Bass()
    x = nc.dram_tensor("x", [128, 256, 256], F16, kind="ExternalInput")
    m = nc.dram_tensor("m", [128, 128], F16, kind="ExternalInput")
    # y dims: [p=(kD mu), chunk, (kH kW q r)]
    y = nc.dram_tensor("y", [128, 8, 8192], F16, kind="ExternalOutput")

    with tile.TileContext(nc) as tc:
        with (
            tc.tile_pool(name="consts", bufs=1) as cpool,
            tc.tile_pool(name="xin", bufs=3) as xpool,
            tc.tile_pool(name="wb", bufs=3) as wpool,
            tc.tile_pool(name="hb", bufs=3) as hpool,
            tc.tile_pool(name="yout", bufs=3) as ypool,
            tc.tile_pool(name="ps", bufs=2, space="PSUM") as pspool,
        ):
            Mt = cpool.tile([128, 128], F16)

            def run_chunks():
                for c in range(8):
                    # input chunk split across both HWDGE rings (SP 26 rows,
                    # Act 6) to balance ring occupancy
                    X = xpool.tile([128, 8192], F16, tag="X")
                    Xr = X[:].rearrange("p (h w) -> p h w", h=32)
                    nc.sync.dma_start(
                        out=Xr[:, 0:26],
                        in_=x[:, 32 * c : 32 * c + 26, :],
                    )
                    nc.scalar.dma_start(
                        out=Xr[:, 26:32],
                        in_=x[:, 32 * c + 26 : 32 * c + 32, :],
                    )
                    if c == 0:
                        # M load after the first X DMA: not needed until the
                        # first matmul, keeps chunk 0 off the critical path
                        nc.sync.dma_start(out=Mt[:], in_=m[:])

                    # W-axis: pairs along w (stride-2) -> (kW, h, r).
                    # Pool only: it is stride-insensitive (0.83 ns/elem)
                    # and cannot touch PSUM, so it gets the strided stage.
                    Wb = wpool.tile([128, 8192], F16, tag="W")
                    Xv = X[:].rearrange("p (h r two) -> p h r two", h=32, two=2)
                    Wv = Wb[:].rearrange("p (kW h r) -> p kW h r", kW=2, h=32)
                    nc.gpsimd.tensor_add(
                        out=Wv[:, 0], in0=Xv[:, :, :, 0], in1=Xv[:, :, :, 1]
                    )
                    nc.gpsimd.tensor_sub(
                        out=Wv[:, 1], in0=Xv[:, :, :, 0], in1=Xv[:, :, :, 1]
                    )

                    # H-axis: pairs along h (h = 2q + b) -> (kH, kW, q, r).
                    # Contiguous 128-elem runs: DVE fast path takes 13 of 16
                    # q-rows, Pool the rest.
                    Hb = hpool.tile([128, 8192], F16, tag="H")
                    Wp = Wb[:].rearrange(
                        "p (kW q b r) -> p kW q b r", kW=2, q=16, b=2
                    )
                    Hv = Hb[:].rearrange(
                        "p (kH kW q r) -> p kH kW q r", kH=2, kW=2, q=16
                    )
                    qs = 13  # DVE q-rows
                    for kH, op_v, op_p in (
                        (0, nc.vector.tensor_add, nc.gpsimd.tensor_add),
                        (1, nc.vector.tensor_sub, nc.gpsimd.tensor_sub),
                    ):
                        op_v(
                            out=Hv[:, kH, :, 0:qs],
                            in0=Wp[:, :, 0:qs, 0],
                            in1=Wp[:, :, 0:qs, 1],
                        )
                        op_p(
                            out=Hv[:, kH, :, qs:16],
                            in0=Wp[:, :, qs:16, 0],
                            in1=Wp[:, :, qs:16, 1],
                        )

                    # D-axis on PE (scale folded into Mt) + psum->SBUF casts
                    # spread across Act / Pool / DVE. Out-DMAs are staged so
                    # each starts as soon as its quarters are cast: Act ring
                    # writes t0-t1 then t2, SP ring (idle late) takes t3.
                    Yt = ypool.tile([128, 8192], F16, tag="Y")
                    for t in range(4):
                        ps = pspool.tile([128, 2048], F32, tag="ps")
                        for j in range(4):
                            s = 512 * j
                            nc.tensor.matmul(
                                ps[:, s : s + 512],
                                Mt[:],
                                Hb[:, 2048 * t + s : 2048 * t + s + 512],
                                start=True,
                                stop=True,
                            )
                        dst = Yt[:, 2048 * t : 2048 * (t + 1)]
                        if t < 2:
                            nc.scalar.copy(out=dst, in_=ps[:])
                        else:
                            nc.vector.tensor_copy(out=dst, in_=ps[:])
                        if t == 1:
                            nc.scalar.dma_start(
                                out=y[:, c, 0:4096], in_=Yt[:, 0:4096]
                            )
                        elif t == 2:
                            nc.sync.dma_start(
                                out=y[:, c, 4096:6144], in_=Yt[:, 4096:6144]
                            )
                        elif t == 3:
                            nc.sync.dma_start(
                                out=y[:, c, 6144:8192], in_=Yt[:, 6144:8192]
                            )

            if reps == 1:
                run_chunks()
            else:
                with tc.For_i(0, reps, 1):
                    run_chunks()

    if reps == 1:
        _PROGRAM = nc
    return nc


def kernel(x: np.ndarray):
    x = np.asarray(x)
    assert x.shape == (2, 4, 128, 256, 256)
    nc = _build_program()

    m = _build_haar_matrix()
    xs = x.reshape(8, 128, 256, 256).astype(np.float16)
    in_maps = [{"x": np.ascontiguousarray(xs[i]), "m": m} for i in range(8)]
    res = bass_utils.run_bass_kernel_spmd(
        nc, in_maps, core_ids=list(range(8)), trace=False
    )

    bands = np.empty((8, 2, 4, 64, 128, 128), np.float32)
    for i in range(8):
        yc = res.results[i]["y"].reshape(2, 64, 8, 2, 2, 16, 128)
        # dims (kD, mu, c, kH, kW, q, r) -> (kD, kH, kW, mu, (c q), r)
        bands[:, i // 4, i % 4] = (
            yc.transpose(0, 3, 4, 1, 2, 5, 6)
            .reshape(8, 64, 128, 128)
            .astype(np.float32)
        )
    return tuple(bands[s] for s in range(8))
